# revision 3
# baseline (speedup 1.0000x reference)
"""GCN message-passing kernel v2 for Trainium2, sharded over 8 NeuronCores.

Design (vs v1 baseline):
- bf16 gather tables with 256B rows; per-edge messages bulk-gathered with
  the SWDGE dma_gather ucode (256B/edge is the HW floor).
- Scatter-add = PE matmul with the 128-edge message chunk as the
  STATIONARY operand ([128e, F] -> cheap LDWEIGHTS) and a per-chunk
  one-hot dest matrix S [128e, 128d] bf16 as the MOVING operand; output
  accumulates feature-major [F, 128d] in PSUM.
- Edges ordered (block of NB dest tiles, window, tile) so each dest
  tile's PSUM bank accumulates across all 4 source windows; one ACT
  copy PSUM->SBUF per dest tile per layer. No DVE adds, no acc memset.
- S built per chunk with one DVE tensor_scalar is_equal (bf16 4x mode).
- Tables built node-major directly (xT tile stationary, W moving): no
  PE transposes. dis_src norm folded into x on host (xsT = (dis*x).T).
  Biases are zero in this problem; a K=1 matmul path handles nonzero.
- Per-window table tensors so layer gathers depend only on their
  window's writes (table build pipelines with aggregation).
- Layer 1 -> 2 handoff: in-place dis^2*relu on the feature-major acc,
  one bf16 AllGather, layer-2 table built from it.
- Pooling: per-tile PE transpose to node-major + one-hot batch matmul
  accumulated in one PSUM bank; tiny AllReduce; replicated head.
"""

import sys

for _p in ("/opt/trn_rl_repo",):
    if _p not in sys.path:
        sys.path.insert(0, _p)

import os

import numpy as np

N = 100000
D_IN = 128
H1 = 32
H2 = 64
NCLS = 10
NG = 64
NCORES = 8
NLOC_REAL = 12500
NLOC = 12544          # per-core padded node count (98 * 128)
NP = NLOC * NCORES    # 100352 padded total
TLOC = NLOC // 128    # 98 dest tiles per core
WIN = 32768
NWIN = (NP + WIN - 1) // WIN  # 4
NTILES_G = NP // 128  # 784 global node tiles
CALL_CHUNKS = int(__import__("os").environ.get("GNN_CC", "32"))  # max chunks per dma_gather call
NB = 5                # dest tiles per PSUM block (<= 6 banks in flight)


def _win_len(w):
    return min(WIN, NP - w * WIN)


def _host_prep(x, edge_index, batch):
    x = np.asarray(x, np.float32)
    ei = np.asarray(edge_index)
    batch64 = np.asarray(batch, np.int64)

    row = ei[0].astype(np.int64)
    col = ei[1].astype(np.int64)
    deg = (np.bincount(row, minlength=N) + 1).astype(np.float32)  # +1 self loop

    def newid(g):
        return (g // NLOC_REAL) * NLOC + (g % NLOC_REAL)

    # self loops appended as ordinary edges
    loops = np.arange(N, dtype=np.int64)
    row = np.concatenate([row, loops])
    col = np.concatenate([col, loops])
    nrow = newid(row)
    ncol = newid(col)

    deg_new = np.ones(NP, np.float32)
    nid = newid(np.arange(N))
    deg_new[nid] = deg
    dis_new = deg_new ** -0.5

    # xsT = (dis * x) padded, transposed, bf16
    xp = np.zeros((NP, D_IN), np.float32)
    xp[nid] = x * dis_new[nid][:, None]
    import ml_dtypes
    bf16 = ml_dtypes.bfloat16
    xsT = np.ascontiguousarray(xp.T).astype(bf16)  # [128, NP]

    cnt = np.bincount(batch64, minlength=NG).astype(np.float32)
    cntr = (1.0 / np.maximum(cnt, 1.0)).reshape(NG, 1)

    # per-core edge structures, sorted by (block, window, tile)
    ecore = ncol // NLOC
    percore = []
    for c in range(NCORES):
        m = ecore == c
        cr = nrow[m]
        cc = ncol[m] - c * NLOC
        w = (cr // WIN).astype(np.int64)
        t = (cc >> 7).astype(np.int64)
        dr = (cc & 127).astype(np.int64)
        blk = t // NB
        order = np.lexsort((cr, t, w, blk))  # source-sorted within segment
        percore.append((cr[order], w[order], t[order], dr[order]))

    # unified chunk counts K[w][t] (max over cores -> SPMD-identical program)
    K = np.zeros((NWIN, TLOC), np.int64)
    seg_counts = []
    for c in range(NCORES):
        cr, w, t, dr = percore[c]
        key = w * TLOC + t
        cnts = np.bincount(key, minlength=NWIN * TLOC).reshape(NWIN, TLOC)
        seg_counts.append(cnts)
        K = np.maximum(K, (cnts + 127) // 128)
    assert (K.sum(axis=0) > 0).all()  # every tile has edges (self loops)

    NBLK = (TLOC + NB - 1) // NB
    # segment emission order: block-major, then window, then tile
    # segments: (blk, w, t, q0, nk, start, stop) with q0 in chunk units
    segments = []
    q = 0
    TK = K.sum(axis=0)          # total chunks per tile
    done = np.zeros(TLOC, np.int64)
    for b in range(NBLK):
        tiles = range(b * NB, min((b + 1) * NB, TLOC))
        for w in range(NWIN):
            for t in tiles:
                nk = int(K[w, t])
                while nk > 0:
                    piece = min(nk, CALL_CHUNKS)
                    st = done[t] == 0
                    done[t] += piece
                    sp = done[t] == TK[t]
                    segments.append((b, w, t, q, piece, bool(st), bool(sp)))
                    q += piece
                    nk -= piece
    NCH = q
    EPAD = NCH * 128
    assert (done == TK).all()

    # gather calls: group consecutive segments within one (blk, w)
    calls = []  # [blk, w, q_start, nq, [seg...]]
    cur = None
    for seg in segments:
        b, w, q0, nk = seg[0], seg[1], seg[3], seg[4]
        if cur is not None and cur[0] == b and cur[1] == w and cur[3] + nk <= CALL_CHUNKS:
            cur[3] += nk
            cur[4].append(seg)
        else:
            if cur is not None:
                calls.append(cur)
            cur = [b, w, q0, nk, [seg]]
    if cur is not None:
        calls.append(cur)

    # per-core padded idx (window-relative) and dest_rel arrays in the
    # unified segment order
    idxw_list, drp_list = [], []
    for c in range(NCORES):
        cr, w, t, dr = percore[c]
        cnts = seg_counts[c]
        # edges already sorted in (blk, w, t) order; bucket starts:
        starts = {}
        pos = 0
        for b in range(NBLK):
            for wi in range(NWIN):
                for ti in range(b * NB, min((b + 1) * NB, TLOC)):
                    starts[(wi, ti)] = pos
                    pos += int(cnts[wi, ti])
        assert pos == len(cr)

        idx_flat = np.zeros(EPAD, np.int16)
        dr_flat = np.full(EPAD, -1.0, np.float32)
        consumed = {k: 0 for k in starts}
        for (b, wi, ti, q0, nk, st, sp) in segments:
            n_total = int(cnts[wi, ti])
            c0 = consumed[(wi, ti)]
            n = min(nk * 128, n_total - c0)
            if n > 0:
                i0 = starts[(wi, ti)] + c0
                o0 = q0 * 128
                idx_flat[o0:o0 + n] = (cr[i0:i0 + n] - wi * WIN).astype(np.int16)
                dr_flat[o0:o0 + n] = dr[i0:i0 + n].astype(np.float32)
                consumed[(wi, ti)] += n
        for (wi, ti), v in consumed.items():
            assert v == int(cnts[wi, ti]), (c, wi, ti, v)

        if os.environ.get("GNN_NOPERM") is None:
            # per-chunk permutation: SDMA engine e serves chunk positions
            # {4a..4a+3, 4a+32..4a+35} (a per the s2m swizzle); give each
            # engine a contiguous run of 8 source-sorted edges so its HBM
            # reads are ascending and close together.
            swz_first = np.array([0, 64, 4, 68, 8, 72, 12, 76,
                                  16, 80, 20, 84, 24, 88, 28, 92])
            perm_p = np.empty(128, np.int64)
            for r in range(16):
                a = swz_first[r]
                perm_p[8 * r:8 * r + 4] = a + np.arange(4)
                perm_p[8 * r + 4:8 * r + 8] = a + 32 + np.arange(4)
            ic = idx_flat.reshape(NCH, 128)
            dc = dr_flat.reshape(NCH, 128)
            o = np.argsort(ic, axis=1, kind="stable")
            ics = np.take_along_axis(ic, o, axis=1)
            dcs = np.take_along_axis(dc, o, axis=1)
            ic[:, perm_p] = ics
            dc[:, perm_p] = dcs

        wrapped = idx_flat.reshape(EPAD // 16, 16).T.copy()  # [16, EPAD//16]
        idxw_list.append(np.tile(wrapped, (8, 1)))            # [128, EPAD//16]
        drp_list.append(np.ascontiguousarray(
            dr_flat.reshape(NCH, 128).T))                     # [128, NCH] f32

    # per-core local metadata
    dis1r_list, dis2r_list, bsel_list = [], [], []
    for c in range(NCORES):
        dl = dis_new[c * NLOC:(c + 1) * NLOC]
        dis1r_list.append(np.broadcast_to(
            dl.astype(bf16), (H2, NLOC)).copy())              # [64, NLOC]
        dis2r_list.append(np.broadcast_to(
            (dl * dl).astype(bf16), (H1, NLOC)).copy())       # [32, NLOC]
        bo = np.full(NLOC, -1, np.int64)
        g0 = c * NLOC_REAL
        bo[:NLOC_REAL] = batch64[g0:g0 + NLOC_REAL]
        bsel = np.zeros((NLOC, NG), np.float32)
        real = bo >= 0
        bsel[np.nonzero(real)[0], bo[real]] = 1.0
        # [128, TLOC, NG] -> [128, TLOC*NG]
        bsel_list.append(np.ascontiguousarray(
            bsel.reshape(TLOC, 128, NG).transpose(1, 0, 2)
            .reshape(128, TLOC * NG)).astype(bf16))

    return dict(xsT=xsT, cntr=cntr, idxw=idxw_list, drp=drp_list,
                dis1r=dis1r_list, dis2r=dis2r_list, bsel=bsel_list,
                NCH=NCH, segments=segments, calls=calls)


def _build_program(NCH, segments, calls):
    import concourse.bacc as bacc
    import concourse.mybir as mybir
    import concourse.tile as tile
    from concourse import library_config

    f32 = mybir.dt.float32
    bf = mybir.dt.bfloat16
    i16 = mybir.dt.int16
    AF = mybir.ActivationFunctionType
    OP = mybir.AluOpType

    nswq = int(os.environ.get("GNN_NSWQ", "4"))
    scratch = int(os.environ.get("GNN_SCRATCH", "32768"))
    nc = bacc.Bacc("TRN2", target_bir_lowering=False, debug=False,
                   num_devices=NCORES, num_swdge_queues=nswq,
                   dynamic_dma_scratch_size=scratch)

    # I/O
    xsT = nc.dram_tensor("xsT", [128, NP], bf, kind="ExternalInput")
    idxw = nc.dram_tensor("idxw", [128, NCH * 8], i16, kind="ExternalInput")
    drp = nc.dram_tensor("drp", [128, NCH], f32, kind="ExternalInput")
    dis1rd = nc.dram_tensor("dis1r", [H2, NLOC], bf, kind="ExternalInput")
    dis2rd = nc.dram_tensor("dis2r", [H1, NLOC], bf, kind="ExternalInput")
    bseld = nc.dram_tensor("bsel", [128, TLOC * NG], bf, kind="ExternalInput")
    cntrd = nc.dram_tensor("cntr", [NG, 1], f32, kind="ExternalInput")
    w1t = nc.dram_tensor("w1t", [D_IN, H1], bf, kind="ExternalInput")
    w2t = nc.dram_tensor("w2t", [H1, H2], bf, kind="ExternalInput")
    wft = nc.dram_tensor("wft", [H2, NCLS], f32, kind="ExternalInput")
    rampd = nc.dram_tensor("ramp", [128, 128], bf, kind="ExternalInput")
    identd = nc.dram_tensor("ident", [128, 128], bf, kind="ExternalInput")
    identf = nc.dram_tensor("identf", [NG, NG], f32, kind="ExternalInput")
    y = nc.dram_tensor("y", [NG, NCLS], f32, kind="ExternalOutput")

    # internal DRAM: per-window gather tables (bf16, 256B rows)
    tab1w = [nc.dram_tensor(f"tab1w{w}", [_win_len(w), 128], bf)
             for w in range(NWIN)]
    tab2w = [nc.dram_tensor(f"tab2w{w}", [_win_len(w), 128], bf)
             for w in range(NWIN)]
    cc1_in = nc.dram_tensor("cc1_in", [H1, NLOC], bf)
    cc1_out = nc.dram_tensor("cc1_out", [H1 * NCORES, NLOC], bf,
                             addr_space="Shared")
    cc2_in = nc.dram_tensor("cc2_in", [NG, H2], f32)
    cc2_out = nc.dram_tensor("cc2_out", [NG, H2], f32, addr_space="Shared")

    tab1v = [t.ap().rearrange("(a p) f -> p a f", p=128) for t in tab1w]
    tab2v = [t.ap().rearrange("(a p) f -> p a f", p=128) for t in tab2w]

    rg = [list(range(NCORES))]
    NBLK = (TLOC + NB - 1) // NB
    stages = os.environ.get("GNN_STAGES", "BCDEFG")

    with tile.TileContext(nc) as tc:
        nc.gpsimd.load_library(library_config.mlp)

        with tc.tile_pool(name="const", bufs=1) as cpool:
            ramp = cpool.tile([128, 128], bf)
            nc.sync.dma_start(out=ramp[:], in_=rampd[:])
            ident = cpool.tile([128, 128], bf)
            nc.sync.dma_start(out=ident[:], in_=identd[:])
            idf = cpool.tile([NG, NG], f32)
            nc.sync.dma_start(out=idf[:], in_=identf[:])
            drt = cpool.tile([128, NCH], f32)
            nc.sync.dma_start(out=drt[:], in_=drp[:])
            drtb = cpool.tile([128, NCH], bf)
            nc.vector.tensor_copy(drtb[:], drt[:])
            w1s = cpool.tile([D_IN, H1], bf)
            nc.sync.dma_start(out=w1s[:], in_=w1t[:])
            w2s = cpool.tile([H1, H2], bf)
            nc.sync.dma_start(out=w2s[:], in_=w2t[:])
            wfs = cpool.tile([H2, NCLS], f32)
            nc.sync.dma_start(out=wfs[:], in_=wft[:])
            cnts = cpool.tile([NG, 1], f32)
            nc.sync.dma_start(out=cnts[:], in_=cntrd[:])

            acc1 = cpool.tile([H1, NLOC], bf)
            acc2 = cpool.tile([H2, NLOC], bf)

            def build_table1():
                # tab1 rows = node-major (dis*x) @ W1.T, bf16
                GT = 8  # node tiles per group
                with tc.tile_pool(name="tb1", bufs=3) as pool, \
                     tc.tile_pool(name="tb1p", bufs=2, space="PSUM") as pp:
                    for g0 in range(0, NTILES_G, GT):
                        ng = min(GT, NTILES_G - g0)
                        xt = pool.tile([128, GT * 128], bf, tag="xt")
                        nc.sync.dma_start(
                            out=xt[:, 0:ng * 128],
                            in_=xsT[:, g0 * 128:(g0 + ng) * 128])
                        ps = pp.tile([128, GT, H1], f32, tag="ps")
                        for j in range(ng):
                            nc.tensor.matmul(
                                ps[:, j, :], xt[:, j * 128:(j + 1) * 128],
                                w1s[:], start=True, stop=True)
                        tt = pool.tile([128, GT, H1], bf, tag="tt")
                        nc.scalar.activation(tt[:, 0:ng, :], ps[:, 0:ng, :],
                                             AF.Copy)
                        w = (g0 * 128) // WIN
                        a0 = g0 - (w * WIN) // 128
                        assert (g0 + ng - 1) * 128 < (w + 1) * WIN or \
                            g0 * 128 >= w * WIN
                        nc.sync.dma_start(
                            out=tab1v[w][:, a0:a0 + ng, 0:H1],
                            in_=tt[:, 0:ng, :])

            def build_table2():
                # tab2 rows = node-major h1gathered @ W2.T, bf16
                GT = 8
                with tc.tile_pool(name="tb2", bufs=3) as pool, \
                     tc.tile_pool(name="tb2p", bufs=2, space="PSUM") as pp:
                    for g0 in range(0, NTILES_G, GT):
                        ng = min(GT, NTILES_G - g0)
                        b = g0 // TLOC
                        t0 = g0 - b * TLOC
                        ht = pool.tile([H1, GT * 128], bf, tag="ht")
                        # group may straddle a core boundary; split the DMA
                        n1 = min(ng, TLOC - t0)
                        nc.sync.dma_start(
                            out=ht[:, 0:n1 * 128],
                            in_=cc1_out[b * H1:(b + 1) * H1,
                                        t0 * 128:(t0 + n1) * 128])
                        if n1 < ng:
                            n2 = ng - n1
                            nc.sync.dma_start(
                                out=ht[:, n1 * 128:ng * 128],
                                in_=cc1_out[(b + 1) * H1:(b + 2) * H1,
                                            0:n2 * 128])
                        ps = pp.tile([128, GT, H2], f32, tag="ps")
                        for j in range(ng):
                            nc.tensor.matmul(
                                ps[:, j, :], ht[:, j * 128:(j + 1) * 128],
                                w2s[:], start=True, stop=True)
                        tt = pool.tile([128, GT, H2], bf, tag="tt")
                        nc.scalar.activation(tt[:, 0:ng, :], ps[:, 0:ng, :],
                                             AF.Copy)
                        w = (g0 * 128) // WIN
                        a0 = g0 - (w * WIN) // 128
                        nc.sync.dma_start(
                            out=tab2v[w][:, a0:a0 + ng, 0:H2],
                            in_=tt[:, 0:ng, :])

            def aggregate(tabv, Fl, acc):
                aggmode = os.environ.get("GNN_AGGMODE", "full")
                sbuild = os.environ.get("GNN_SBUILD", "call")
                spkt = bool(int(os.environ.get("GNN_SPKT", "0")))
                gbufs = int(os.environ.get("GNN_GBUFS", "6"))
                CC = max(c[3] for c in calls)
                qrot = [0]
                with tc.tile_pool(name="agg", bufs=gbufs) as gpool, \
                     tc.tile_pool(name="aggi", bufs=gbufs) as ipool, \
                     tc.tile_pool(name="aggs", bufs=12) as spool, \
                     tc.tile_pool(name="aggsc", bufs=4) as scpool, \
                     tc.tile_pool(name="aggp", bufs=6, space="PSUM") as pp:
                    ci = 0
                    for b in range(NBLK):
                        tiles = list(range(b * NB, min((b + 1) * NB, TLOC)))
                        pst = {t: pp.tile([H2, 128], f32, tag="aggps",
                                          name=f"aggps{t}")
                               for t in tiles}
                        for w in range(NWIN):
                            while ci < len(calls) and calls[ci][0] == b \
                                    and calls[ci][1] == w:
                                blk, wi, qs, nq, segs = calls[ci]
                                ci += 1
                                wl = _win_len(wi)
                                it = ipool.tile([128, CC * 8], i16,
                                                tag="it")
                                nc.sync.dma_start(
                                    out=it[:, 0:nq * 8],
                                    in_=idxw[:, qs * 8:(qs + nq) * 8])
                                gb = gpool.tile([128, CC, 128], bf,
                                                tag="gb")
                                nc.gpsimd.dma_gather(
                                    gb[:, 0:nq, :], tabv[wi].ap(),
                                    it[:, 0:nq * 8], nq * 128, nq * 128, 128,
                                    single_packet=spkt,
                                    queue_num=qrot[0] % nswq)
                                qrot[0] += 1
                                if aggmode == "gather":
                                    continue
                                if sbuild == "call":
                                    Sc = scpool.tile(
                                        [128, CC, 128], bf,
                                        tag="Sc")
                                    ramp_b = ramp[:].unsqueeze(1) \
                                        .broadcast_to([128, nq, 128])
                                    dr_b = drtb[:, qs:qs + nq].unsqueeze(2) \
                                        .broadcast_to([128, nq, 128])
                                    nc.vector.tensor_tensor(
                                        Sc[:, 0:nq, :], ramp_b, dr_b,
                                        OP.is_equal)
                                for (_b, _w, t, q0, nk, st, sp) in segs:
                                    for k in range(nk):
                                        q = q0 + k
                                        if sbuild == "call":
                                            Sk = Sc[:, q - qs, :]
                                        else:
                                            Skt = spool.tile([128, 128], bf,
                                                             tag="S")
                                            nc.vector.tensor_scalar(
                                                Skt[:], ramp[:],
                                                drt[:, q:q + 1],
                                                None, OP.is_equal)
                                            Sk = Skt[:]
                                        nc.tensor.matmul(
                                            pst[t][0:Fl, :],
                                            gb[:, q - qs, 0:Fl], Sk,
                                            start=(st and k == 0),
                                            stop=(sp and k == nk - 1))
                        if aggmode == "full":
                            for t in tiles:
                                nc.scalar.activation(
                                    acc[0:Fl, t * 128:(t + 1) * 128],
                                    pst[t][0:Fl, :], AF.Copy)
                    assert ci == len(calls)

            def handoff():
                # acc1 = relu(dis^2 * acc1) feature-major; allgather
                with tc.tile_pool(name="ho", bufs=1) as pool:
                    d2 = pool.tile([H1, NLOC], bf)
                    nc.sync.dma_start(out=d2[:], in_=dis2rd[:])
                    nc.vector.tensor_tensor(acc1[:], acc1[:], d2[:], OP.mult)
                    nc.scalar.activation(acc1[:], acc1[:], AF.Relu)
                    nc.sync.dma_start(out=cc1_in[:, :], in_=acc1[:])
                    if os.environ.get("GNN_NOCOLL"):
                        for b2_ in range(NCORES):
                            nc.sync.dma_start(
                                out=cc1_out[b2_ * H1:(b2_ + 1) * H1, :],
                                in_=cc1_in[:, :])
                    else:
                        nc.gpsimd.collective_compute(
                            "AllGather", OP.bypass, replica_groups=rg,
                            ins=[cc1_in.ap().opt()], outs=[cc1_out.ap().opt()])

            def pool_and_head():
                with tc.tile_pool(name="hd", bufs=3) as pool, \
                     tc.tile_pool(name="hdp", bufs=3, space="PSUM") as pp, \
                     tc.tile_pool(name="hdq", bufs=1, space="PSUM") as pq:
                    d1 = pool.tile([H2, NLOC], bf, tag="d1")
                    nc.sync.dma_start(out=d1[:], in_=dis1rd[:])
                    bss = pool.tile([128, TLOC * NG], bf, tag="bss")
                    nc.sync.dma_start(out=bss[:], in_=bseld[:])
                    nc.vector.tensor_tensor(acc2[:], acc2[:], d1[:], OP.mult)
                    nc.scalar.activation(acc2[:], acc2[:], AF.Relu)
                    pls = pq.tile([NG, H2], f32, tag="pool")
                    for t in range(TLOC):
                        pT = pp.tile([128, H2], bf, tag="pT")
                        nc.tensor.transpose(
                            pT[:], acc2[:, t * 128:(t + 1) * 128],
                            ident[0:H2, 0:H2])
                        hnm = pool.tile([128, H2], bf, tag="hnm")
                        nc.scalar.activation(hnm[:], pT[:], AF.Copy)
                        nc.tensor.matmul(
                            pls[:], bss[:, t * NG:(t + 1) * NG], hnm[:],
                            start=(t == 0), stop=(t == TLOC - 1))
                    pools = pool.tile([NG, H2], f32, tag="pools")
                    nc.scalar.activation(pools[:], pls[:], AF.Copy)
                    nc.sync.dma_start(out=cc2_in[:, :], in_=pools[:])
                    if os.environ.get("GNN_NOCOLL"):
                        nc.sync.dma_start(out=cc2_out[:, :], in_=cc2_in[:, :])
                    else:
                        nc.gpsimd.collective_compute(
                            "AllReduce", OP.add, replica_groups=rg,
                            ins=[cc2_in.ap().opt()], outs=[cc2_out.ap().opt()])
                    psb = pool.tile([NG, H2], f32, tag="psb")
                    nc.sync.dma_start(out=psb[:], in_=cc2_out[:, :])
                    mean = pool.tile([NG, H2], f32, tag="mean")
                    nc.vector.tensor_scalar(mean[:], psb[:], cnts[:], None,
                                            OP.mult)
                    pmT = pp.tile([H2, NG], f32, tag="pmT", bufs=1)
                    nc.tensor.transpose(pmT[:], mean[:], idf[:])
                    meanT = pool.tile([H2, NG], f32, tag="meanT")
                    nc.scalar.activation(meanT[:], pmT[:], AF.Copy)
                    plg = pp.tile([NG, NCLS], f32, tag="plg", bufs=1)
                    nc.tensor.matmul(plg[:], meanT[:], wfs[:],
                                     start=True, stop=True)
                    lg = pool.tile([NG, NCLS], f32, tag="lg")
                    nc.scalar.activation(lg[:], plg[:], AF.Copy)
                    mx = pool.tile([NG, 1], f32, tag="mx")
                    nc.vector.tensor_reduce(mx[:], lg[:], mybir.AxisListType.X,
                                            OP.max, negate=True)
                    ex = pool.tile([NG, NCLS], f32, tag="ex")
                    nc.scalar.activation(ex[:], lg[:], AF.Exp, bias=mx[:])
                    sm = pool.tile([NG, 1], f32, tag="sm")
                    nc.vector.tensor_reduce(sm[:], ex[:], mybir.AxisListType.X,
                                            OP.add)
                    rs = pool.tile([NG, 1], f32, tag="rs")
                    nc.vector.reciprocal(rs[:], sm[:])
                    yt = pool.tile([NG, NCLS], f32, tag="yt")
                    nc.vector.tensor_scalar(yt[:], ex[:], rs[:], None, OP.mult)
                    nc.sync.dma_start(out=y[:, :], in_=yt[:])

            if "B" in stages:
                build_table1()
            if "C" in stages:
                aggregate(tab1w, H1, acc1)
            if "D" in stages:
                handoff()
            if "E" in stages:
                build_table2()
            if "F" in stages:
                aggregate(tab2w, H2, acc2)
            if "G" in stages:
                pool_and_head()
            else:
                with tc.tile_pool(name="dbg", bufs=1) as dpool:
                    dt_ = dpool.tile([NG, NCLS], f32)
                    nc.vector.memset(dt_[:], 0.0)
                    nc.sync.dma_start(out=y[:, :], in_=dt_[:])

    nc.compile()
    return nc


def _make_in_maps(prep, W1, W2, Wf):
    import ml_dtypes
    bf16 = ml_dtypes.bfloat16
    if os.environ.get("GNN_ZEROIDX"):
        prep = dict(prep)
        prep["idxw"] = [np.zeros_like(a) for a in prep["idxw"]]
    ramp = np.tile(np.arange(128, dtype=np.float32), (128, 1)).astype(bf16)
    ident = np.eye(128, dtype=np.float32).astype(bf16)
    identf = np.eye(NG, dtype=np.float32)
    common = dict(
        xsT=prep["xsT"], cntr=prep["cntr"],
        w1t=np.ascontiguousarray(np.asarray(W1, np.float32).T).astype(bf16),
        w2t=np.ascontiguousarray(np.asarray(W2, np.float32).T).astype(bf16),
        wft=np.ascontiguousarray(np.asarray(Wf, np.float32).T),
        ramp=ramp, ident=ident, identf=identf,
    )
    in_maps = []
    for c in range(NCORES):
        m = dict(common)
        m["idxw"] = prep["idxw"][c]
        m["drp"] = prep["drp"][c]
        m["dis1r"] = prep["dis1r"][c]
        m["dis2r"] = prep["dis2r"][c]
        m["bsel"] = prep["bsel"][c]
        in_maps.append(m)
    return in_maps


def kernel(x, edge_index, batch, W1, b1, W2, b2, Wf, bf):
    from concourse.bass_utils import run_bass_kernel_spmd

    assert not np.asarray(b1).any() and not np.asarray(b2).any() \
        and not np.asarray(bf).any(), "nonzero biases not supported"
    prep = _host_prep(x, edge_index, batch)
    nc = _build_program(prep["NCH"], prep["segments"], prep["calls"])
    in_maps = _make_in_maps(prep, W1, W2, Wf)
    res = run_bass_kernel_spmd(nc, in_maps, core_ids=list(range(NCORES)))
    return np.asarray(res.results[0]["y"], np.float32)


# revision 4
# speedup vs baseline: 1.2516x; 1.2516x over previous
"""GCN message-passing kernel v2 for Trainium2, sharded over 8 NeuronCores.

Design (vs v1 baseline):
- bf16 gather tables with 256B rows; per-edge messages bulk-gathered with
  the SWDGE dma_gather ucode (256B/edge is the HW floor).
- Scatter-add = PE matmul with the 128-edge message chunk as the
  STATIONARY operand ([128e, F] -> cheap LDWEIGHTS) and a per-chunk
  one-hot dest matrix S [128e, 128d] bf16 as the MOVING operand; output
  accumulates feature-major [F, 128d] in PSUM.
- Edges ordered (block of NB dest tiles, window, tile) so each dest
  tile's PSUM bank accumulates across all 4 source windows; one ACT
  copy PSUM->SBUF per dest tile per layer. No DVE adds, no acc memset.
- S built per chunk with one DVE tensor_scalar is_equal (bf16 4x mode).
- Tables built node-major directly (xT tile stationary, W moving): no
  PE transposes. dis_src norm folded into x on host (xsT = (dis*x).T).
  Biases are zero in this problem; a K=1 matmul path handles nonzero.
- Per-window table tensors so layer gathers depend only on their
  window's writes (table build pipelines with aggregation).
- Layer 1 -> 2 handoff: in-place dis^2*relu on the feature-major acc,
  one bf16 AllGather, layer-2 table built from it.
- Pooling: per-tile PE transpose to node-major + one-hot batch matmul
  accumulated in one PSUM bank; tiny AllReduce; replicated head.
"""

import sys

for _p in ("/opt/trn_rl_repo",):
    if _p not in sys.path:
        sys.path.insert(0, _p)

import os

import numpy as np

N = 100000
D_IN = 128
H1 = 32
H2 = 64
NCLS = 10
NG = 64
NCORES = 8
NLOC_REAL = 12500
NLOC = 12544          # per-core padded node count (98 * 128)
NP = NLOC * NCORES    # 100352 padded total
TLOC = NLOC // 128    # 98 dest tiles per core
WIN = 32768
NWIN = (NP + WIN - 1) // WIN  # 4
NTILES_G = NP // 128  # 784 global node tiles
CALL_CHUNKS = int(__import__("os").environ.get("GNN_CC", "32"))  # max chunks per dma_gather call
NB = 5                # dest tiles per PSUM block (<= 6 banks in flight)


def _win_len(w):
    return min(WIN, NP - w * WIN)


def _host_prep(x, edge_index, batch):
    x = np.asarray(x, np.float32)
    ei = np.asarray(edge_index)
    batch64 = np.asarray(batch, np.int64)

    row = ei[0].astype(np.int64)
    col = ei[1].astype(np.int64)
    deg = (np.bincount(row, minlength=N) + 1).astype(np.float32)  # +1 self loop

    def newid(g):
        return (g // NLOC_REAL) * NLOC + (g % NLOC_REAL)

    # self loops appended as ordinary edges
    loops = np.arange(N, dtype=np.int64)
    row = np.concatenate([row, loops])
    col = np.concatenate([col, loops])
    nrow = newid(row)
    ncol = newid(col)

    deg_new = np.ones(NP, np.float32)
    nid = newid(np.arange(N))
    deg_new[nid] = deg
    dis_new = deg_new ** -0.5

    # xsT = (dis * x) padded, transposed, bf16
    xp = np.zeros((NP, D_IN), np.float32)
    xp[nid] = x * dis_new[nid][:, None]
    import ml_dtypes
    bf16 = ml_dtypes.bfloat16
    xsT = np.ascontiguousarray(xp.T).astype(bf16)  # [128, NP]

    cnt = np.bincount(batch64, minlength=NG).astype(np.float32)
    cntr = (1.0 / np.maximum(cnt, 1.0)).reshape(NG, 1)

    # per-core edge structures, sorted by (block, window, tile)
    ecore = ncol // NLOC
    percore = []
    for c in range(NCORES):
        m = ecore == c
        cr = nrow[m]
        cc = ncol[m] - c * NLOC
        w = (cr // WIN).astype(np.int64)
        t = (cc >> 7).astype(np.int64)
        dr = (cc & 127).astype(np.int64)
        blk = t // NB
        order = np.lexsort((cr, t, w, blk))  # source-sorted within segment
        percore.append((cr[order], w[order], t[order], dr[order]))

    # unified chunk counts K[w][t] (max over cores -> SPMD-identical program)
    K = np.zeros((NWIN, TLOC), np.int64)
    seg_counts = []
    for c in range(NCORES):
        cr, w, t, dr = percore[c]
        key = w * TLOC + t
        cnts = np.bincount(key, minlength=NWIN * TLOC).reshape(NWIN, TLOC)
        seg_counts.append(cnts)
        K = np.maximum(K, (cnts + 127) // 128)
    assert (K.sum(axis=0) > 0).all()  # every tile has edges (self loops)

    NBLK = (TLOC + NB - 1) // NB
    # segment emission order: block-major, then window, then tile
    # segments: (blk, w, t, q0, nk, start, stop) with q0 in chunk units
    segments = []
    q = 0
    TK = K.sum(axis=0)          # total chunks per tile
    done = np.zeros(TLOC, np.int64)
    for b in range(NBLK):
        tiles = range(b * NB, min((b + 1) * NB, TLOC))
        for w in range(NWIN):
            for t in tiles:
                nk = int(K[w, t])
                while nk > 0:
                    piece = min(nk, CALL_CHUNKS)
                    st = done[t] == 0
                    done[t] += piece
                    sp = done[t] == TK[t]
                    segments.append((b, w, t, q, piece, bool(st), bool(sp)))
                    q += piece
                    nk -= piece
    NCH = q
    EPAD = NCH * 128
    assert (done == TK).all()

    # gather calls: group consecutive segments within one (blk, w)
    calls = []  # [blk, w, q_start, nq, [seg...]]
    cur = None
    for seg in segments:
        b, w, q0, nk = seg[0], seg[1], seg[3], seg[4]
        if cur is not None and cur[0] == b and cur[1] == w and cur[3] + nk <= CALL_CHUNKS:
            cur[3] += nk
            cur[4].append(seg)
        else:
            if cur is not None:
                calls.append(cur)
            cur = [b, w, q0, nk, [seg]]
    if cur is not None:
        calls.append(cur)

    # per-core padded idx (window-relative) and dest_rel arrays in the
    # unified segment order
    idxw_list, drp_list = [], []
    for c in range(NCORES):
        cr, w, t, dr = percore[c]
        cnts = seg_counts[c]
        # edges already sorted in (blk, w, t) order; bucket starts:
        starts = {}
        pos = 0
        for b in range(NBLK):
            for wi in range(NWIN):
                for ti in range(b * NB, min((b + 1) * NB, TLOC)):
                    starts[(wi, ti)] = pos
                    pos += int(cnts[wi, ti])
        assert pos == len(cr)

        idx_flat = np.zeros(EPAD, np.int16)
        dr_flat = np.full(EPAD, -1.0, np.float32)
        consumed = {k: 0 for k in starts}
        for (b, wi, ti, q0, nk, st, sp) in segments:
            n_total = int(cnts[wi, ti])
            c0 = consumed[(wi, ti)]
            n = min(nk * 128, n_total - c0)
            if n > 0:
                i0 = starts[(wi, ti)] + c0
                o0 = q0 * 128
                idx_flat[o0:o0 + n] = (cr[i0:i0 + n] - wi * WIN).astype(np.int16)
                dr_flat[o0:o0 + n] = dr[i0:i0 + n].astype(np.float32)
                consumed[(wi, ti)] += n
        for (wi, ti), v in consumed.items():
            assert v == int(cnts[wi, ti]), (c, wi, ti, v)

        if os.environ.get("GNN_NOPERM") is None:
            # per-chunk permutation: SDMA engine e serves chunk positions
            # {4a..4a+3, 4a+32..4a+35} (a per the s2m swizzle); give each
            # engine a contiguous run of 8 source-sorted edges so its HBM
            # reads are ascending and close together.
            swz_first = np.array([0, 64, 4, 68, 8, 72, 12, 76,
                                  16, 80, 20, 84, 24, 88, 28, 92])
            perm_p = np.empty(128, np.int64)
            for r in range(16):
                a = swz_first[r]
                perm_p[8 * r:8 * r + 4] = a + np.arange(4)
                perm_p[8 * r + 4:8 * r + 8] = a + 32 + np.arange(4)
            ic = idx_flat.reshape(NCH, 128)
            dc = dr_flat.reshape(NCH, 128)
            o = np.argsort(ic, axis=1, kind="stable")
            ics = np.take_along_axis(ic, o, axis=1)
            dcs = np.take_along_axis(dc, o, axis=1)
            ic[:, perm_p] = ics
            dc[:, perm_p] = dcs

        wrapped = idx_flat.reshape(EPAD // 16, 16).T.copy()  # [16, EPAD//16]
        idxw_list.append(np.tile(wrapped, (8, 1)))            # [128, EPAD//16]
        drp_list.append(np.ascontiguousarray(
            dr_flat.reshape(NCH, 128).T))                     # [128, NCH] f32

    # per-core local metadata
    dis1r_list, dis2r_list, bsel_list = [], [], []
    for c in range(NCORES):
        dl = dis_new[c * NLOC:(c + 1) * NLOC]
        dis1r_list.append(np.broadcast_to(
            dl.astype(bf16), (H2, NLOC)).copy())              # [64, NLOC]
        dis2r_list.append(np.broadcast_to(
            (dl * dl).astype(bf16), (H1, NLOC)).copy())       # [32, NLOC]
        bo = np.full(NLOC, -1, np.int64)
        g0 = c * NLOC_REAL
        bo[:NLOC_REAL] = batch64[g0:g0 + NLOC_REAL]
        bsel = np.zeros((NLOC, NG), np.float32)
        real = bo >= 0
        bsel[np.nonzero(real)[0], bo[real]] = 1.0
        # [128, TLOC, NG] -> [128, TLOC*NG]
        bsel_list.append(np.ascontiguousarray(
            bsel.reshape(TLOC, 128, NG).transpose(1, 0, 2)
            .reshape(128, TLOC * NG)).astype(bf16))

    return dict(xsT=xsT, cntr=cntr, idxw=idxw_list, drp=drp_list,
                dis1r=dis1r_list, dis2r=dis2r_list, bsel=bsel_list,
                NCH=NCH, segments=segments, calls=calls)


def _build_program(NCH, segments, calls):
    import concourse.bacc as bacc
    import concourse.mybir as mybir
    import concourse.tile as tile
    from concourse import library_config

    f32 = mybir.dt.float32
    bf = mybir.dt.bfloat16
    i16 = mybir.dt.int16
    AF = mybir.ActivationFunctionType
    OP = mybir.AluOpType

    nswq = int(os.environ.get("GNN_NSWQ", "4"))
    scratch = int(os.environ.get("GNN_SCRATCH", "32768"))
    nc = bacc.Bacc("TRN2", target_bir_lowering=False, debug=False,
                   num_devices=NCORES, num_swdge_queues=nswq,
                   dynamic_dma_scratch_size=scratch)

    # I/O
    xsT = nc.dram_tensor("xsT", [128, NP], bf, kind="ExternalInput")
    idxw = nc.dram_tensor("idxw", [128, NCH * 8], i16, kind="ExternalInput")
    drp = nc.dram_tensor("drp", [128, NCH], f32, kind="ExternalInput")
    dis1rd = nc.dram_tensor("dis1r", [H2, NLOC], bf, kind="ExternalInput")
    dis2rd = nc.dram_tensor("dis2r", [H1, NLOC], bf, kind="ExternalInput")
    bseld = nc.dram_tensor("bsel", [128, TLOC * NG], bf, kind="ExternalInput")
    cntrd = nc.dram_tensor("cntr", [NG, 1], f32, kind="ExternalInput")
    w1t = nc.dram_tensor("w1t", [D_IN, H1], bf, kind="ExternalInput")
    w2t = nc.dram_tensor("w2t", [H1, H2], bf, kind="ExternalInput")
    wft = nc.dram_tensor("wft", [H2, NCLS], f32, kind="ExternalInput")
    rampd = nc.dram_tensor("ramp", [128, 128], bf, kind="ExternalInput")
    identd = nc.dram_tensor("ident", [128, 128], bf, kind="ExternalInput")
    identf = nc.dram_tensor("identf", [NG, NG], f32, kind="ExternalInput")
    y = nc.dram_tensor("y", [NG, NCLS], f32, kind="ExternalOutput")

    # internal DRAM: per-window gather tables (bf16, 256B rows)
    tab1w = [nc.dram_tensor(f"tab1w{w}", [_win_len(w), 128], bf)
             for w in range(NWIN)]
    tab2w = [nc.dram_tensor(f"tab2w{w}", [_win_len(w), 128], bf)
             for w in range(NWIN)]
    cc1_in = nc.dram_tensor("cc1_in", [H1, NLOC], bf)
    cc1_out = nc.dram_tensor("cc1_out", [H1 * NCORES, NLOC], bf,
                             addr_space="Shared")
    cc2_in = nc.dram_tensor("cc2_in", [NG, H2], f32)
    cc2_out = nc.dram_tensor("cc2_out", [NG, H2], f32, addr_space="Shared")

    tab1v = [t.ap().rearrange("(a p) f -> p a f", p=128) for t in tab1w]
    tab2v = [t.ap().rearrange("(a p) f -> p a f", p=128) for t in tab2w]

    rg = [list(range(NCORES))]
    NBLK = (TLOC + NB - 1) // NB
    stages = os.environ.get("GNN_STAGES", "BCDEFG")

    with tile.TileContext(nc) as tc:
        nc.gpsimd.load_library(library_config.mlp)

        with tc.tile_pool(name="const", bufs=1) as cpool:
            ramp = cpool.tile([128, 128], bf)
            nc.sync.dma_start(out=ramp[:], in_=rampd[:])
            ident = cpool.tile([128, 128], bf)
            nc.sync.dma_start(out=ident[:], in_=identd[:])
            idf = cpool.tile([NG, NG], f32)
            nc.sync.dma_start(out=idf[:], in_=identf[:])
            drt = cpool.tile([128, NCH], f32)
            nc.sync.dma_start(out=drt[:], in_=drp[:])
            drtb = cpool.tile([128, NCH], bf)
            nc.vector.tensor_copy(drtb[:], drt[:])
            w1s = cpool.tile([D_IN, H1], bf)
            nc.sync.dma_start(out=w1s[:], in_=w1t[:])
            w2s = cpool.tile([H1, H2], bf)
            nc.sync.dma_start(out=w2s[:], in_=w2t[:])
            wfs = cpool.tile([H2, NCLS], f32)
            nc.sync.dma_start(out=wfs[:], in_=wft[:])
            cnts = cpool.tile([NG, 1], f32)
            nc.sync.dma_start(out=cnts[:], in_=cntrd[:])

            acc1 = cpool.tile([H1, NLOC], bf)
            acc2 = cpool.tile([H2, NLOC], bf)

            def build_table1():
                # tab1 rows = node-major (dis*x) @ W1.T, bf16
                GT = 8  # node tiles per group
                with tc.tile_pool(name="tb1", bufs=3) as pool, \
                     tc.tile_pool(name="tb1p", bufs=2, space="PSUM") as pp:
                    for g0 in range(0, NTILES_G, GT):
                        ng = min(GT, NTILES_G - g0)
                        xt = pool.tile([128, GT * 128], bf, tag="xt")
                        nc.sync.dma_start(
                            out=xt[:, 0:ng * 128],
                            in_=xsT[:, g0 * 128:(g0 + ng) * 128])
                        ps = pp.tile([128, GT, H1], f32, tag="ps")
                        for j in range(ng):
                            nc.tensor.matmul(
                                ps[:, j, :], xt[:, j * 128:(j + 1) * 128],
                                w1s[:], start=True, stop=True)
                        tt = pool.tile([128, GT, H1], bf, tag="tt")
                        nc.scalar.activation(tt[:, 0:ng, :], ps[:, 0:ng, :],
                                             AF.Copy)
                        w = (g0 * 128) // WIN
                        a0 = g0 - (w * WIN) // 128
                        assert (g0 + ng - 1) * 128 < (w + 1) * WIN or \
                            g0 * 128 >= w * WIN
                        nc.sync.dma_start(
                            out=tab1v[w][:, a0:a0 + ng, 0:H1],
                            in_=tt[:, 0:ng, :])

            def build_table2():
                # tab2 rows = node-major h1gathered @ W2.T, bf16
                GT = 8
                with tc.tile_pool(name="tb2", bufs=3) as pool, \
                     tc.tile_pool(name="tb2p", bufs=2, space="PSUM") as pp:
                    for g0 in range(0, NTILES_G, GT):
                        ng = min(GT, NTILES_G - g0)
                        b = g0 // TLOC
                        t0 = g0 - b * TLOC
                        ht = pool.tile([H1, GT * 128], bf, tag="ht")
                        # group may straddle a core boundary; split the DMA
                        n1 = min(ng, TLOC - t0)
                        nc.sync.dma_start(
                            out=ht[:, 0:n1 * 128],
                            in_=cc1_out[b * H1:(b + 1) * H1,
                                        t0 * 128:(t0 + n1) * 128])
                        if n1 < ng:
                            n2 = ng - n1
                            nc.sync.dma_start(
                                out=ht[:, n1 * 128:ng * 128],
                                in_=cc1_out[(b + 1) * H1:(b + 2) * H1,
                                            0:n2 * 128])
                        ps = pp.tile([128, GT, H2], f32, tag="ps")
                        for j in range(ng):
                            nc.tensor.matmul(
                                ps[:, j, :], ht[:, j * 128:(j + 1) * 128],
                                w2s[:], start=True, stop=True)
                        tt = pool.tile([128, GT, H2], bf, tag="tt")
                        nc.scalar.activation(tt[:, 0:ng, :], ps[:, 0:ng, :],
                                             AF.Copy)
                        w = (g0 * 128) // WIN
                        a0 = g0 - (w * WIN) // 128
                        nc.sync.dma_start(
                            out=tab2v[w][:, a0:a0 + ng, 0:H2],
                            in_=tt[:, 0:ng, :])

            def aggregate(tabv, Fl, acc):
                aggmode = os.environ.get("GNN_AGGMODE", "full")
                sbuild = os.environ.get("GNN_SBUILD", "call")
                spkt = bool(int(os.environ.get("GNN_SPKT", "0")))
                gbufs = int(os.environ.get("GNN_GBUFS", "6"))
                CC = max(c[3] for c in calls)
                qrot = [0]
                with tc.tile_pool(name="agg", bufs=gbufs) as gpool, \
                     tc.tile_pool(name="aggi", bufs=gbufs) as ipool, \
                     tc.tile_pool(name="aggs", bufs=12) as spool, \
                     tc.tile_pool(name="aggsc", bufs=4) as scpool, \
                     tc.tile_pool(name="aggp",
                                  bufs=int(os.environ.get("GNN_PBUFS", "6")),
                                  space="PSUM") as pp:
                    ci = 0
                    for b in range(NBLK):
                        tiles = list(range(b * NB, min((b + 1) * NB, TLOC)))
                        pst = {t: pp.tile([H2, 128], f32, tag="aggps",
                                          name=f"aggps{t}")
                               for t in tiles}
                        for w in range(NWIN):
                            while ci < len(calls) and calls[ci][0] == b \
                                    and calls[ci][1] == w:
                                blk, wi, qs, nq, segs = calls[ci]
                                ci += 1
                                wl = _win_len(wi)
                                it = ipool.tile([128, CC * 8], i16,
                                                tag="it")
                                nc.sync.dma_start(
                                    out=it[:, 0:nq * 8],
                                    in_=idxw[:, qs * 8:(qs + nq) * 8])
                                gb = gpool.tile([128, CC, 128], bf,
                                                tag="gb")
                                nc.gpsimd.dma_gather(
                                    gb[:, 0:nq, :], tabv[wi].ap(),
                                    it[:, 0:nq * 8], nq * 128, nq * 128, 128,
                                    single_packet=spkt,
                                    queue_num=qrot[0] % nswq)
                                qrot[0] += 1
                                if aggmode == "gather":
                                    continue
                                if sbuild == "call":
                                    Sc = scpool.tile(
                                        [128, CC, 128], bf,
                                        tag="Sc")
                                    ramp_b = ramp[:].unsqueeze(1) \
                                        .broadcast_to([128, nq, 128])
                                    dr_b = drtb[:, qs:qs + nq].unsqueeze(2) \
                                        .broadcast_to([128, nq, 128])
                                    nc.vector.tensor_tensor(
                                        Sc[:, 0:nq, :], ramp_b, dr_b,
                                        OP.is_equal)
                                for (_b, _w, t, q0, nk, st, sp) in segs:
                                    for k in range(nk):
                                        q = q0 + k
                                        if sbuild == "call":
                                            Sk = Sc[:, q - qs, :]
                                        else:
                                            Skt = spool.tile([128, 128], bf,
                                                             tag="S")
                                            nc.vector.tensor_scalar(
                                                Skt[:], ramp[:],
                                                drt[:, q:q + 1],
                                                None, OP.is_equal)
                                            Sk = Skt[:]
                                        nc.tensor.matmul(
                                            pst[t][0:Fl, :],
                                            gb[:, q - qs, 0:Fl], Sk,
                                            start=(st and k == 0),
                                            stop=(sp and k == nk - 1))
                        if aggmode == "full":
                            for t in tiles:
                                nc.scalar.activation(
                                    acc[0:Fl, t * 128:(t + 1) * 128],
                                    pst[t][0:Fl, :], AF.Copy)
                    assert ci == len(calls)

            def handoff():
                # acc1 = relu(dis^2 * acc1) feature-major; allgather
                with tc.tile_pool(name="ho", bufs=1) as pool:
                    d2 = pool.tile([H1, NLOC], bf)
                    nc.sync.dma_start(out=d2[:], in_=dis2rd[:])
                    nc.vector.tensor_tensor(acc1[:], acc1[:], d2[:], OP.mult)
                    nc.scalar.activation(acc1[:], acc1[:], AF.Relu)
                    nc.sync.dma_start(out=cc1_in[:, :], in_=acc1[:])
                    if os.environ.get("GNN_NOCOLL"):
                        for b2_ in range(NCORES):
                            nc.sync.dma_start(
                                out=cc1_out[b2_ * H1:(b2_ + 1) * H1, :],
                                in_=cc1_in[:, :])
                    else:
                        nc.gpsimd.collective_compute(
                            "AllGather", OP.bypass, replica_groups=rg,
                            ins=[cc1_in.ap().opt()], outs=[cc1_out.ap().opt()])

            def pool_and_head():
                with tc.tile_pool(name="hd", bufs=3) as pool, \
                     tc.tile_pool(name="hdp", bufs=3, space="PSUM") as pp, \
                     tc.tile_pool(name="hdq", bufs=1, space="PSUM") as pq:
                    d1 = pool.tile([H2, NLOC], bf, tag="d1")
                    nc.sync.dma_start(out=d1[:], in_=dis1rd[:])
                    bss = pool.tile([128, TLOC * NG], bf, tag="bss")
                    nc.sync.dma_start(out=bss[:], in_=bseld[:])
                    nc.vector.tensor_tensor(acc2[:], acc2[:], d1[:], OP.mult)
                    nc.scalar.activation(acc2[:], acc2[:], AF.Relu)
                    pls = pq.tile([NG, H2], f32, tag="pool")
                    for t in range(TLOC):
                        pT = pp.tile([128, H2], bf, tag="pT")
                        nc.tensor.transpose(
                            pT[:], acc2[:, t * 128:(t + 1) * 128],
                            ident[0:H2, 0:H2])
                        hnm = pool.tile([128, H2], bf, tag="hnm")
                        nc.scalar.activation(hnm[:], pT[:], AF.Copy)
                        nc.tensor.matmul(
                            pls[:], bss[:, t * NG:(t + 1) * NG], hnm[:],
                            start=(t == 0), stop=(t == TLOC - 1))
                    pools = pool.tile([NG, H2], f32, tag="pools")
                    nc.scalar.activation(pools[:], pls[:], AF.Copy)
                    nc.sync.dma_start(out=cc2_in[:, :], in_=pools[:])
                    if os.environ.get("GNN_NOCOLL"):
                        nc.sync.dma_start(out=cc2_out[:, :], in_=cc2_in[:, :])
                    else:
                        nc.gpsimd.collective_compute(
                            "AllReduce", OP.add, replica_groups=rg,
                            ins=[cc2_in.ap().opt()], outs=[cc2_out.ap().opt()])
                    psb = pool.tile([NG, H2], f32, tag="psb")
                    nc.sync.dma_start(out=psb[:], in_=cc2_out[:, :])
                    mean = pool.tile([NG, H2], f32, tag="mean")
                    nc.vector.tensor_scalar(mean[:], psb[:], cnts[:], None,
                                            OP.mult)
                    pmT = pp.tile([H2, NG], f32, tag="pmT", bufs=1)
                    nc.tensor.transpose(pmT[:], mean[:], idf[:])
                    meanT = pool.tile([H2, NG], f32, tag="meanT")
                    nc.scalar.activation(meanT[:], pmT[:], AF.Copy)
                    plg = pp.tile([NG, NCLS], f32, tag="plg", bufs=1)
                    nc.tensor.matmul(plg[:], meanT[:], wfs[:],
                                     start=True, stop=True)
                    lg = pool.tile([NG, NCLS], f32, tag="lg")
                    nc.scalar.activation(lg[:], plg[:], AF.Copy)
                    mx = pool.tile([NG, 1], f32, tag="mx")
                    nc.vector.tensor_reduce(mx[:], lg[:], mybir.AxisListType.X,
                                            OP.max, negate=True)
                    ex = pool.tile([NG, NCLS], f32, tag="ex")
                    nc.scalar.activation(ex[:], lg[:], AF.Exp, bias=mx[:])
                    sm = pool.tile([NG, 1], f32, tag="sm")
                    nc.vector.tensor_reduce(sm[:], ex[:], mybir.AxisListType.X,
                                            OP.add)
                    rs = pool.tile([NG, 1], f32, tag="rs")
                    nc.vector.reciprocal(rs[:], sm[:])
                    yt = pool.tile([NG, NCLS], f32, tag="yt")
                    nc.vector.tensor_scalar(yt[:], ex[:], rs[:], None, OP.mult)
                    nc.sync.dma_start(out=y[:, :], in_=yt[:])

            if "B" in stages:
                build_table1()
            if "C" in stages:
                aggregate(tab1w, H1, acc1)
            if "D" in stages:
                handoff()
            if "E" in stages:
                build_table2()
            if "F" in stages:
                aggregate(tab2w, H2, acc2)
            if "G" in stages:
                pool_and_head()
            else:
                with tc.tile_pool(name="dbg", bufs=1) as dpool:
                    dt_ = dpool.tile([NG, NCLS], f32)
                    nc.vector.memset(dt_[:], 0.0)
                    nc.sync.dma_start(out=y[:, :], in_=dt_[:])

    nc.compile()
    return nc


def _make_in_maps(prep, W1, W2, Wf):
    import ml_dtypes
    bf16 = ml_dtypes.bfloat16
    if os.environ.get("GNN_ZEROIDX"):
        prep = dict(prep)
        prep["idxw"] = [np.zeros_like(a) for a in prep["idxw"]]
    ramp = np.tile(np.arange(128, dtype=np.float32), (128, 1)).astype(bf16)
    ident = np.eye(128, dtype=np.float32).astype(bf16)
    identf = np.eye(NG, dtype=np.float32)
    common = dict(
        xsT=prep["xsT"], cntr=prep["cntr"],
        w1t=np.ascontiguousarray(np.asarray(W1, np.float32).T).astype(bf16),
        w2t=np.ascontiguousarray(np.asarray(W2, np.float32).T).astype(bf16),
        wft=np.ascontiguousarray(np.asarray(Wf, np.float32).T),
        ramp=ramp, ident=ident, identf=identf,
    )
    in_maps = []
    for c in range(NCORES):
        m = dict(common)
        m["idxw"] = prep["idxw"][c]
        m["drp"] = prep["drp"][c]
        m["dis1r"] = prep["dis1r"][c]
        m["dis2r"] = prep["dis2r"][c]
        m["bsel"] = prep["bsel"][c]
        in_maps.append(m)
    return in_maps


def kernel(x, edge_index, batch, W1, b1, W2, b2, Wf, bf):
    from concourse.bass_utils import run_bass_kernel_spmd

    assert not np.asarray(b1).any() and not np.asarray(b2).any() \
        and not np.asarray(bf).any(), "nonzero biases not supported"
    prep = _host_prep(x, edge_index, batch)
    nc = _build_program(prep["NCH"], prep["segments"], prep["calls"])
    in_maps = _make_in_maps(prep, W1, W2, Wf)
    res = run_bass_kernel_spmd(nc, in_maps, core_ids=list(range(NCORES)))
    return np.asarray(res.results[0]["y"], np.float32)
